# revision 31
# baseline (speedup 1.0000x reference)
"""Trainium2 Bass kernel for nn_NodeEdgeConv (GNN message passing).

Strategy (v2, destination-sharded, collective-free):

- Algebraic reduction: segment_sum(h[idx]*msg, idx)[n] = h[n]*segment_sum(msg,idx)[n]
  and segment_sum(v @ W + b) = segment_sum([v|1]) @ [[W],[b]], so the only
  edge-level work is a segment-sum of the [E, 65] payloads (v plus a ones
  column that yields counts); everything else is node-level GEMMs.
- Each core owns 1024 dst nodes + 1024 src nodes (2048 "virtual" rows).
  Host routes every edge (both types) to the owner core of its destination,
  sorts by virtual row, and pads each row's edge list to a multiple of 4.
  No collective is needed.
- Quad pre-reduction: the 4 edges of a quad are stored in two DRAM arrays
  (even/odd pair members). Level-1 pair sums happen inside the DMA (SWDGE
  accum_op=add - same HBM bytes), level-2 on DVE. PE then sees 4x fewer
  rows.
- Segment-sum on PE: for each 128-quad tile, matmul(lhsT=quad_v [128,65]
  bf16, rhs=onehot [128,64] bf16) accumulates into a persistent PSUM
  aggregate aggT' [65, 2048] (has_written semantics give per-element
  overwrite-then-accumulate on a DVE-zeroed bank). Virtual rows are grouped
  into 64-node windows; per-window tile counts are maxed across cores so
  the instruction schedule is SPMD-identical. One-hots are generated by a
  batched DVE is_equal against an iota using stride-0 access patterns.
- Finish (per 512-node chunk, transposed layout [D, nodes]): exact host
  algebra folds remove all broadcasts: W1c = W1 - colmean(W1) makes the
  LayerNorm mean vanish, W2g = diag(g) @ W2 absorbs the gain, and
  b2' = beta @ W2 + b2 is pre-added into the f32 residual embeddings.
  Variance comes from ACT Square + a ones-column matmul; the per-node
  rsqrt is broadcast across partitions with a K=1 matmul. Nonzero biases
  are applied as K=1 rank-1 matmuls (skipped when exactly zero).
"""

import numpy as np

import concourse.bass as bass
import concourse.bacc as bacc
import concourse.mybir as mybir
import concourse.tile as tile

F32 = mybir.dt.float32
BF16 = mybir.dt.bfloat16
BF16_NP = mybir.dt.np(BF16)


class Cfg:
    def __init__(self):
        self.N = 8192          # nodes per side
        self.E = 524288        # edges per type
        self.D = 128
        self.M = 64
        self.C = 8             # cores
        self.NSH = self.N // self.C   # 1024 nodes per side per core
        self.V = 2 * self.NSH          # 2048 virtual rows per core
        self.W = 64            # window (block) width in nodes
        self.NB = self.V // self.W     # 32 blocks per core
        self.CH = 12           # DMA chunks
        self.eps = 1e-5


# ---------------- host-side prep ----------------

def host_prep(inputs, cfg):
    C, NSH, V, W, NB = cfg.C, cfg.NSH, cfg.V, cfg.W, cfg.NB
    E = cfg.E

    idxA = np.asarray(inputs["e_s2d_dst"]).astype(np.int64)
    idxB = np.asarray(inputs["e_d2s_dst"]).astype(np.int64)
    vA = np.asarray(inputs["v_s2d"], dtype=np.float32)
    vB = np.asarray(inputs["v_d2s"], dtype=np.float32)

    core = np.concatenate([idxA // NSH, idxB // NSH])
    vn = np.concatenate([idxA % NSH, idxB % NSH + NSH])
    gkey = core * V + vn                       # [2E] in [0, C*V)

    order = np.argsort(gkey, kind="stable")
    gs = gkey[order]
    cnt = np.bincount(gkey, minlength=C * V)   # edges per (core, vn)
    qn = (cnt + 3) // 4                        # quads per (core, vn)

    # shared block tiling: tiles per block = max over cores
    qblk = qn.reshape(C, NB, W).sum(-1)        # [C, NB] quads per block
    Tb = np.maximum((qblk.max(0) + 127) // 128, 1)  # [NB]
    Bb = np.concatenate([[0], np.cumsum(Tb)])  # tile base per block
    T_real = int(Bb[-1])
    CHT = (T_real + cfg.CH - 1) // cfg.CH
    T = CHT * cfg.CH
    blk_of_tile = np.empty(T, np.int64)
    for b in range(NB):
        blk_of_tile[Bb[b]:Bb[b + 1]] = b
    blk_of_tile[T_real:] = NB - 1              # pad tiles -> last block

    # quad slot for each sorted edge
    starts = np.concatenate([[0], np.cumsum(cnt)])
    rank = np.arange(2 * E) - starts[gs]
    sub = rank & 3
    qw = rank >> 2
    # quad offset of each (core, vn) within its block
    csq = np.cumsum(qn)
    key_first = (np.arange(C)[:, None] * V + np.arange(NB)[None, :] * W).ravel()
    blk_q0 = (csq[key_first] - qn[key_first]).reshape(C, NB)
    qoff = (csq - qn) - blk_q0[
        np.arange(C * V) // V, (np.arange(C * V) % V) // W]
    s = Bb[(gs % V) // W] * 128 + qoff[gs] + qw   # slot in core's stream
    t_of = s // 128
    p_of = s % 128
    u2 = (sub >> 1)

    vcat = np.concatenate([vA, vB])[order].astype(BF16_NP)
    cedge = core[order]

    # E0 = quad members 0,2 ; E1 = members 1,3  (E1 DMA-accumulates onto E0
    # in SBUF -> pair sums); within-record layout [2, 64]
    E0 = np.zeros((C, 128, T, 2, 64), BF16_NP)
    E1 = np.zeros((C, 128, T, 2, 64), BF16_NP)
    half = (sub >> 1)
    is_odd = (sub & 1) == 1
    m0 = ~is_odd
    E0[cedge[m0], p_of[m0], t_of[m0], half[m0]] = vcat[m0]
    E1[cedge[is_odd], p_of[is_odd], t_of[is_odd], half[is_odd]] = vcat[is_odd]

    # per-quad real-edge count (replaces the ones column)
    qcnt = np.zeros((C, 128, T), np.float32)
    np.add.at(qcnt, (cedge, p_of, t_of), 1.0)
    qcnt = qcnt.astype(BF16_NP)

    # c values (node offset within the tile's 64-node window)
    cval = np.zeros((C, 128, T), BF16_NP)
    cv = (vn[order] % W).astype(np.float32)
    cval[cedge, p_of, t_of] = cv.astype(BF16_NP)

    # sanity: every real quad's window holds it
    assert (cv >= 0).all() and (cv < W).all()

    # ---- weights / algebra folds (exact, fp64 where it matters) ----
    def g(name):
        return np.asarray(inputs[name], dtype=np.float64)

    sides = {}
    for s_, pre, Wh_n, bh_n, Wm_n, bm_n in (
        (0, "col", "W_dst", "b_dst", "W_sm", "b_sm"),
        (1, "row", "W_src", "b_src", "W_dm", "b_dm"),
    ):
        W1 = g(f"{pre}_W1")
        b1 = g(f"{pre}_b1")
        gn = g(f"{pre}_g")
        be = g(f"{pre}_beta")
        W2 = g(f"{pre}_W2")
        b2 = g(f"{pre}_b2")
        w1m = W1.mean(axis=1)
        W1c = W1 - w1m[:, None]
        c1 = b1 - b1.mean()
        W2g = gn[:, None] * W2
        b2p = be @ W2 + b2
        Wmp = np.concatenate([g(Wm_n), g(bm_n)[None, :]], axis=0)  # [65,128]
        sides[s_] = dict(
            Wh=g(Wh_n).astype(BF16_NP), bh=g(bh_n).astype(BF16_NP),
            Wmp=Wmp.astype(BF16_NP), W1c=W1c.astype(BF16_NP),
            c1=c1.astype(BF16_NP), W2g=W2g.astype(BF16_NP),
            b2p=b2p, bh_zero=not np.any(g(bh_n)), c1_zero=not np.any(c1),
        )

    semb = np.asarray(inputs["src_embed"], dtype=np.float32)
    demb = np.asarray(inputs["dst_embed"], dtype=np.float32)

    sched = dict(
        T=T, CHT=CHT, blk=blk_of_tile,
        bh_zero=(sides[0]["bh_zero"], sides[1]["bh_zero"]),
        c1_zero=(sides[0]["c1_zero"], sides[1]["c1_zero"]),
    )

    # packed bf16 const layout (columns):
    #   embTb [V] | cval [T] | qcnt [T] | per side: Wh, W1c, W2g, Wmp (128 ea)
    #   | optional bh0|c10|bh1|c11 (1 col each, on partition 0 ... row vecs
    #     are [1,128] so store them as one 128-col block each)
    pk_cols = V + 8 * 128
    off = {}
    o = 0
    off["embTb"] = o; o += V
    for s_ in (0, 1):
        for nm in ("Wh", "W1c", "W2g", "Wmp"):
            off[f"{nm}{s_}"] = o; o += 128
    extra = []
    for s_ in (0, 1):
        if not sides[s_]["bh_zero"]:
            off[f"bh{s_}"] = o; o += 128; extra.append((f"bh{s_}", "bh"))
        if not sides[s_]["c1_zero"]:
            off[f"c1{s_}"] = o; o += 128; extra.append((f"c1{s_}", "c1"))
    pk_cols = o
    sched["pk_off"] = off
    sched["pk_cols"] = pk_cols

    in_maps = []
    for c in range(C):
        nsl = slice(c * NSH, (c + 1) * NSH)
        embT = np.concatenate([demb[nsl].T, semb[nsl].T], axis=1)  # [128,2048]
        embTaug = embT.astype(np.float64).copy()
        embTaug[:, :NSH] += sides[0]["b2p"][:, None]
        embTaug[:, NSH:] += sides[1]["b2p"][:, None]
        pk = np.zeros((128, pk_cols), BF16_NP)
        pk[:, off["embTb"]:off["embTb"] + V] = embT.astype(BF16_NP)
        pk2 = np.zeros((128, 2 * T), BF16_NP)
        pk2[:, :T] = cval[c]
        pk2[:, T:] = qcnt[c]
        for s_ in (0, 1):
            sd = sides[s_]
            pk[:, off[f"Wh{s_}"]:off[f"Wh{s_}"] + 128] = sd["Wh"]
            pk[:, off[f"W1c{s_}"]:off[f"W1c{s_}"] + 128] = sd["W1c"]
            pk[:, off[f"W2g{s_}"]:off[f"W2g{s_}"] + 128] = sd["W2g"]
            pk[:65, off[f"Wmp{s_}"]:off[f"Wmp{s_}"] + 128] = sd["Wmp"]
            if not sd["bh_zero"]:
                pk[0, off[f"bh{s_}"]:off[f"bh{s_}"] + 128] = sd["bh"]
            if not sd["c1_zero"]:
                pk[0, off[f"c1{s_}"]:off[f"c1{s_}"] + 128] = sd["c1"]
        m = {
            "E0": np.ascontiguousarray(E0[c].reshape(128, T * 128)),
            "E1": np.ascontiguousarray(E1[c].reshape(128, T * 128)),
            "ohp": np.ascontiguousarray(
                (cval[c][:, :, None].astype(np.float32)
                 == np.arange(W, dtype=np.float32)[None, None, :]
                 ).astype(BF16_NP).reshape(128, T * W)),
            "pk": pk,
            "pk2": pk2,
            "embTaug": np.ascontiguousarray(embTaug.astype(np.float32)),
        }
        in_maps.append(m)
    return in_maps, sched


# ---------------- device kernel ----------------

def build_kernel(cfg, sched, dbg_skip_finish=False, dbg_skip_agg=False,
                 num_devices=None):
    C, V, W, NB, CH = cfg.C, cfg.V, cfg.W, cfg.NB, cfg.CH
    T, CHT, blk = sched["T"], sched["CHT"], sched["blk"]
    NSH = cfg.NSH

    nc = bacc.Bacc("TRN2", target_bir_lowering=False, debug=False,
                   num_devices=num_devices or C)

    pk_off, pk_cols = sched["pk_off"], sched["pk_cols"]

    E0 = nc.dram_tensor("E0", [128, T * 128], BF16, kind="ExternalInput")
    E1 = nc.dram_tensor("E1", [128, T * 128], BF16, kind="ExternalInput")
    ohp = nc.dram_tensor("ohp", [128, T * W], BF16, kind="ExternalInput")
    pk = nc.dram_tensor("pk", [128, pk_cols], BF16, kind="ExternalInput")
    pk2 = nc.dram_tensor("pk2", [128, 2 * T], BF16, kind="ExternalInput")
    embTaug = nc.dram_tensor("embTaug", [128, V], F32, kind="ExternalInput")
    outT = nc.dram_tensor("outT", [128, V], F32, kind="ExternalOutput")

    # finish chunk f emitted after the last tile of block 8f+7
    last_tile_of_block = np.zeros(NB, np.int64)
    for t_ in range(T):
        last_tile_of_block[blk[t_]] = t_
    fin_after = {}
    pb = 512 // W
    for f in range(4):
        fin_after[int(last_tile_of_block[(f + 1) * pb - 1])] = f

    with tile.TileContext(nc) as tc:
        with (
            nc.allow_low_precision(reason="bf16 pre-reduction is within the "
                                   "2e-2 tolerance budget"),
            tc.tile_pool(name="const", bufs=1) as const,
            tc.tile_pool(name="io", bufs=4) as io,
            tc.tile_pool(name="fin", bufs=2) as fin,
            tc.tile_pool(name="ps", bufs=1, space="PSUM") as ps,
            tc.tile_pool(name="psf", bufs=2, space="PSUM") as psf,
        ):
            # ---- constants ----
            iota = const.tile([128, W], BF16, tag="iota")
            nc.gpsimd.iota(iota[:], pattern=[[1, W]], base=0,
                           channel_multiplier=0,
                           allow_small_or_imprecise_dtypes=True)
            ones_row = const.tile([1, 512], BF16, tag="ones_row")
            nc.vector.memset(ones_row[:], 1.0)
            ones_col = const.tile([128, 1], BF16, tag="ones_col")
            nc.vector.memset(ones_col[:], 1.0 / 128.0)
            eps_t = const.tile([1, 1], F32, tag="eps")
            nc.vector.memset(eps_t[:], float(cfg.eps))
            ones_1x128 = const.tile([1, 128], BF16, tag="ones_1x")
            nc.vector.memset(ones_1x128[:], 1.0)

            pk2_t = const.tile([128, 2 * T], BF16, tag="pk2")
            nc.scalar.dma_start(pk2_t[:], pk2.ap())
            pk_t = const.tile([128, pk_cols], BF16, tag="pk")
            nc.scalar.dma_start(pk_t[:], pk.ap())
            embTaug_t = const.tile([128, V], F32, tag="embTaug")
            nc.scalar.dma_start(embTaug_t[:], embTaug.ap())

            # resident quad-value and one-hot planes
            qv_r = const.tile([128, T * 65], BF16, tag="qv_r")
            oh_r = const.tile([128, T * W], BF16, tag="oh_r")
            qap = qv_r[:]
            qcout = bass.AP(tensor=qap.tensor, offset=qap.offset + 64,
                            ap=[list(qap.ap[0]), [65, T]])
            nc.vector.tensor_copy(qcout, pk2_t[:, T:2 * T])

            wts = {}
            for s_ in (0, 1):
                for nm in ("Wh", "W1c", "W2g"):
                    o_ = pk_off[f"{nm}{s_}"]
                    wts[f"{nm}{s_}"] = pk_t[:, o_:o_ + 128]
                o_ = pk_off[f"Wmp{s_}"]
                wts[f"Wmp{s_}"] = pk_t[:65, o_:o_ + 128]
                if not sched["bh_zero"][s_]:
                    o_ = pk_off[f"bh{s_}"]
                    wts[f"bh{s_}"] = pk_t[:1, o_:o_ + 128]
                if not sched["c1_zero"][s_]:
                    o_ = pk_off[f"c1{s_}"]
                    wts[f"c1{s_}"] = pk_t[:1, o_:o_ + 128]
            embTb_t = pk_t[:, pk_off["embTb"]:pk_off["embTb"] + V]
            out_sb = const.tile([128, V], F32, tag="out_sb")

            # ---- persistent PSUM aggregate [65, 2048] as 4 banks ----
            agg = [ps.tile([65, 512], F32, tag=f"agg{i}", name=f"agg{i}")
                   for i in range(4)]
            for a in agg:
                nc.vector.memset(a[:], 0.0)

            def emit_finish(f):
                s_ = f // 2
                csl = slice(f * 512, (f + 1) * 512)
                ebsl = slice(pk_off["embTb"] + f * 512,
                             pk_off["embTb"] + (f + 1) * 512)
                if dbg_skip_finish:
                    nc.vector.tensor_copy(out_sb[:65, csl], agg[f][:])
                    return
                aggsb = fin.tile([65, 512], BF16, tag="aggsb")
                nc.scalar.activation(
                    aggsb[:], agg[f][:],
                    func=mybir.ActivationFunctionType.Copy)
                ST_ps = psf.tile([128, 512], F32, tag="fs")
                nc.tensor.matmul(ST_ps[:], lhsT=wts[f"Wmp{s_}"],
                                 rhs=aggsb[:])
                ST_sb = fin.tile([128, 512], BF16, tag="ST")
                nc.vector.tensor_copy(ST_sb[:], ST_ps[:])

                hT_ps = psf.tile([128, 512], F32, tag="fs")
                bz = sched["bh_zero"][s_]
                nc.tensor.matmul(hT_ps[:], lhsT=wts[f"Wh{s_}"],
                                 rhs=pk_t[:, ebsl], start=True, stop=bz)
                if not bz:
                    nc.tensor.matmul(hT_ps[:], lhsT=wts[f"bh{s_}"],
                                     rhs=ones_row[:], start=False, stop=True)
                uT_sb = fin.tile([128, 512], BF16, tag="uT")
                nc.vector.tensor_mul(uT_sb[:], hT_ps[:], ST_sb[:])

                t1_ps = psf.tile([128, 512], F32, tag="fs")
                cz = sched["c1_zero"][s_]
                nc.tensor.matmul(t1_ps[:], lhsT=wts[f"W1c{s_}"],
                                 rhs=uT_sb[:], start=True, stop=cz)
                if not cz:
                    nc.tensor.matmul(t1_ps[:], lhsT=wts[f"c1{s_}"],
                                     rhs=ones_row[:], start=False, stop=True)
                sq_sb = fin.tile([128, 512], BF16, tag="sq")
                nc.scalar.activation(
                    sq_sb[:], t1_ps[:],
                    func=mybir.ActivationFunctionType.Square)
                t1_sb = fin.tile([128, 512], BF16, tag="t1")
                nc.scalar.activation(
                    t1_sb[:], t1_ps[:],
                    func=mybir.ActivationFunctionType.Copy)

                # t2raw = W2g.T @ t1 runs in parallel with the rsqrt branch;
                # the per-node LN scale commutes through the matmul.
                t2_ps = psf.tile([128, 512], F32, tag="fs")
                nc.tensor.matmul(t2_ps[:], lhsT=wts[f"W2g{s_}"],
                                 rhs=t1_sb[:])
                t2_sb = fin.tile([128, 512], BF16, tag="t2")
                nc.vector.tensor_copy(t2_sb[:], t2_ps[:])

                ssq_ps = psf.tile([1, 512], F32, tag="fq")
                nc.tensor.matmul(ssq_ps[:], lhsT=ones_col[:], rhs=sq_sb[:])
                sr_sb = fin.tile([1, 512], F32, tag="sr")
                nc.scalar.activation(
                    sr_sb[:], ssq_ps[:],
                    func=mybir.ActivationFunctionType.Sqrt,
                    bias=eps_t[:])
                r_sb = fin.tile([1, 512], BF16, tag="r")
                nc.vector.reciprocal(r_sb[:], sr_sb[:])

                rb_ps = psf.tile([128, 512], F32, tag="fs")
                nc.tensor.matmul(rb_ps[:], lhsT=ones_1x128[:], rhs=r_sb[:])
                hh_sb = fin.tile([128, 512], F32, tag="hh")
                nc.vector.tensor_mul(hh_sb[:], rb_ps[:], t2_sb[:])
                nc.vector.tensor_add(out_sb[:, csl], hh_sb[:],
                                     embTaug_t[:, csl])
                nc.scalar.dma_start(outT.ap()[:, csl], out_sb[:, csl])

            # ---- main aggregation loop ----
            for ch in range(CH):
                fsl = slice(ch * CHT * 128, (ch + 1) * CHT * 128)
                p01 = io.tile([128, CHT * 128], BF16, tag="p01")
                nc.sync.dma_start(p01[:], E0.ap()[:, fsl])
                # SWDGE accumulate crashes above 4KB per partition row;
                # split into <=2048-col pieces.
                for lo in range(0, CHT * 128, 2048):
                    hi = min(lo + 2048, CHT * 128)
                    nc.gpsimd.dma_start(
                        p01[:, lo:hi],
                        E1.ap()[:, ch * CHT * 128 + lo:ch * CHT * 128 + hi],
                        accum_op=mybir.AluOpType.add)
                osl = slice(ch * CHT * W, (ch + 1) * CHT * W)
                nc.scalar.dma_start(
                    bass.AP(tensor=oh_r[:].tensor,
                            offset=oh_r[:].offset + ch * CHT * W,
                            ap=[list(oh_r[:].ap[0]), [1, CHT * W]]),
                    ohp.ap()[:, osl])

                qap = qv_r[:]
                papp = p01[:]
                j0 = bass.AP(tensor=papp.tensor, offset=papp.offset,
                             ap=[list(papp.ap[0]), [128, CHT], [1, 64]])
                j1 = bass.AP(tensor=papp.tensor, offset=papp.offset + 64,
                             ap=[list(papp.ap[0]), [128, CHT], [1, 64]])
                qout = bass.AP(tensor=qap.tensor,
                               offset=qap.offset + ch * CHT * 65,
                               ap=[list(qap.ap[0]), [65, CHT], [1, 64]])
                nc.vector.tensor_tensor(qout, j0, j1, op=mybir.AluOpType.add)

                for tl in range(CHT):
                    t_ = ch * CHT + tl
                    b = int(blk[t_])
                    pb = 512 // W
                    bank, off = b // pb, (b % pb) * W
                    if not dbg_skip_agg:
                        nc.tensor.matmul(
                            agg[bank][:, off:off + W],
                            lhsT=qv_r[:, t_ * 65:(t_ + 1) * 65],
                            rhs=oh_r[:, t_ * W:(t_ + 1) * W],
                            start=False, stop=False, skip_group_check=True)
                    if t_ in fin_after:
                        emit_finish(fin_after[t_])


    nc.compile()
    return nc


# ---------------- host-side assemble ----------------

def assemble(results, cfg):
    NSH = cfg.NSH
    col = np.concatenate(
        [np.ascontiguousarray(r["outT"][:, :NSH].T) for r in results], axis=0)
    row = np.concatenate(
        [np.ascontiguousarray(r["outT"][:, NSH:].T) for r in results], axis=0)
    return row, col


# ---------------- graded entry point ----------------

_CACHE = {}


def kernel(**inputs):
    cfg = Cfg()
    in_maps, sched = host_prep(inputs, cfg)
    key = (sched["T"], sched["CHT"], sched["blk"].tobytes(),
           sched["bh_zero"], sched["c1_zero"])
    if key not in _CACHE:
        _CACHE[key] = build_kernel(cfg, sched)
    nc = _CACHE[key]
    from concourse.bass_utils import run_bass_kernel_spmd
    res = run_bass_kernel_spmd(nc, in_maps, core_ids=list(range(cfg.C)))
    return assemble(res.results, cfg)
